# revision 7
# baseline (speedup 1.0000x reference)
"""Mamba (selective-scan) block at 3 scales on 8 TRN2 NeuronCores.

Sharding: data-parallel over batch (B=8 -> 1 batch element per core).
Per core, per scale, the full mamba_forward runs in (channel, seq) layout:
  x [C, L] -> rmsnorm -> in_proj (PE, bf16) -> causal conv (DVE FMAs)
  -> silu -> x_proj/dt_proj (PE) -> softplus -> selective scan via the
  DVE TensorTensorScan instruction (h_t = dA*h + dBu per (d,n) lane)
  -> C-contraction (bf16 tree add) -> gate -> out_proj (PE) -> residual
  -> final rmsnorm.
"""
import numpy as np
import ml_dtypes
from contextlib import ExitStack

import concourse.bass as bass
import concourse.tile as tile
from concourse import bacc, mybir
from concourse.bass_utils import run_bass_kernel_spmd

F32 = mybir.dt.float32
BF16 = mybir.dt.bfloat16
AF = mybir.ActivationFunctionType
OP = mybir.AluOpType
EPS = 1e-5
NST = 16  # d_state
# (d, H, W, T_chunk)
SCALES = [(256, 64, 64, 512), (512, 32, 32, 512), (1024, 16, 16, 256)]
NCORES = 8


def _bcast_ap(dram_ap, row, cols):
    """AP reading one DRAM row broadcast across 128 partitions."""
    sl = dram_ap[row:row + 1, :cols]
    return bass.AP(tensor=sl.tensor, offset=sl.offset, ap=[[0, 128], sl.ap[-1]])


def _scale_program(nc, tc, ctx, i, d, L, T):
    di, G, KC = 2 * d, 2 * d // 128, d // 128
    r = -(-d // 16)
    b_off = ((r + 31) // 32) * 32
    c_off = b_off + 32
    rpn = 128  # padded x_proj output rows (dt@0, B@b_off, C@c_off)
    NCH = L // T

    x_d = nc.dram_tensor(f"x{i}", [d, L], F32, kind="ExternalInput").ap()
    win_d = nc.dram_tensor(f"win{i}", [d, di * 2], BF16, kind="ExternalInput").ap()
    wx_d = nc.dram_tensor(f"wx{i}", [di, rpn], BF16, kind="ExternalInput").ap()
    wdt_d = nc.dram_tensor(f"wdt{i}", [r, di], BF16, kind="ExternalInput").ap()
    wo_d = nc.dram_tensor(f"wo{i}", [di, d], BF16, kind="ExternalInput").ap()
    convw_d = nc.dram_tensor(f"convw{i}", [128, G * 4], F32, kind="ExternalInput").ap()
    convb_d = nc.dram_tensor(f"convb{i}", [128, G], F32, kind="ExternalInput").ap()
    dtb_d = nc.dram_tensor(f"dtb{i}", [128, G], F32, kind="ExternalInput").ap()
    dd_d = nc.dram_tensor(f"dd{i}", [128, G], F32, kind="ExternalInput").ap()
    nfw_d = nc.dram_tensor(f"nfw{i}", [128, KC], F32, kind="ExternalInput").ap()
    av_d = nc.dram_tensor(f"av{i}", [1, NST], F32, kind="ExternalInput").ap()
    o_d = nc.dram_tensor(f"o{i}", [d, L], F32, kind="ExternalOutput").ap()

    # DRAM scratch for partition-broadcast bounces
    bc_scr = nc.dram_tensor(f"bcscr{i}", [NCH, 2 * NST, T], BF16).ap()
    rs_scr = nc.dram_tensor(f"rsscr{i}", [NCH, 2, T], F32).ap()

    const = ctx.enter_context(tc.tile_pool(name=f"const{i}", bufs=1))
    state = ctx.enter_context(tc.tile_pool(name=f"state{i}", bufs=1))
    lhsp = ctx.enter_context(tc.tile_pool(name=f"lhs{i}", bufs=4))
    xp = ctx.enter_context(tc.tile_pool(name=f"xp{i}", bufs=1))
    rowp = ctx.enter_context(tc.tile_pool(name=f"row{i}", bufs=2))
    gp = ctx.enter_context(tc.tile_pool(name=f"gp{i}", bufs=1))
    convp = ctx.enter_context(tc.tile_pool(name=f"conv{i}", bufs=3))
    scanp = ctx.enter_context(tc.tile_pool(name=f"scan{i}", bufs=4))
    bigp = ctx.enter_context(tc.tile_pool(name=f"big{i}", bufs=1))
    treep = ctx.enter_context(tc.tile_pool(name=f"tree{i}", bufs=1))
    psA = ctx.enter_context(tc.tile_pool(name=f"psA{i}", bufs=3, space="PSUM"))
    psB = ctx.enter_context(tc.tile_pool(name=f"psB{i}", bufs=1, space="PSUM"))
    psS = ctx.enter_context(tc.tile_pool(name=f"psS{i}", bufs=1, space="PSUM"))

    # constants
    convw = const.tile([128, G * 4], F32)
    nc.sync.dma_start(convw, convw_d)
    convb = const.tile([128, G], F32)
    nc.sync.dma_start(convb, convb_d)
    dtb = const.tile([128, G], F32)
    nc.sync.dma_start(dtb, dtb_d)
    dd = const.tile([128, G], F32)
    nc.sync.dma_start(dd, dd_d)
    nfw = const.tile([128, KC], F32)
    nc.sync.dma_start(nfw, nfw_d)
    a_t = const.tile([128, NST], F32)
    nc.sync.dma_start(a_t, _bcast_ap(av_d, 0, NST))
    ones_f = const.tile([128, 1], F32)
    nc.vector.memset(ones_f, 1.0)
    epsc = const.tile([1, 1], F32)
    nc.vector.memset(epsc, EPS)
    wx_sb = const.tile([128, G, rpn], BF16)
    nc.sync.dma_start(wx_sb, wx_d.rearrange("(g p) q -> p g q", p=128))
    wdt_sb = const.tile([r, di], BF16)
    nc.sync.dma_start(wdt_sb, wdt_d)

    # persistent per-scale state
    hstate = [state.tile([128, NST], F32, tag=f"hs{g}", name=f"hs{i}_{g}")
              for g in range(G)]
    halo = [state.tile([128, 4], F32, tag=f"halo{g}", name=f"halo{i}_{g}")
            for g in range(G)]
    for g in range(G):
        nc.vector.memset(halo[g], 0.0)

    for c in range(NCH):
        cs = slice(c * T, (c + 1) * T)
        # ---- rmsnorm of x chunk ----
        x_t, xnb = [], []
        ps_ss = psS.tile([1, T], F32, tag="ss")
        for kc in range(KC):
            xt = xp.tile([128, T], F32, tag=f"x{kc}")
            nc.sync.dma_start(xt, x_d[kc * 128:(kc + 1) * 128, cs])
            x_t.append(xt)
            xsq = gp.tile([128, T], F32, tag="xsq")
            nc.scalar.square(xsq, xt)
            nc.tensor.matmul(ps_ss, ones_f, xsq, start=(kc == 0), stop=(kc == KC - 1))
        rrow = rowp.tile([1, T], F32, tag="rrow")
        # rrow = 1/sqrt(ms*(1/C)+eps)
        nc.scalar.activation(rrow, ps_ss, AF.Sqrt, bias=epsc[:, 0:1], scale=1.0 / d)
        nc.vector.reciprocal(rrow, rrow)
        nc.sync.dma_start(rs_scr[c, 0:1, :], rrow)
        rrep = rowp.tile([128, T], F32, tag="rrep")
        nc.sync.dma_start(rrep, _bcast_ap(rs_scr[c], 0, T))
        for kc in range(KC):
            xn = xp.tile([128, T], BF16, tag=f"xn{kc}")
            nc.vector.tensor_mul(xn, x_t[kc], rrep)
            xnb.append(xn)
        # ---- in_proj ----
        xi_s, sres = [], []
        for d2 in range(2 * G):
            ps = psA.tile([128, T], F32, tag="mm")
            for kc in range(KC):
                lw = lhsp.tile([128, 128], BF16, tag="lw")
                nc.sync.dma_start(
                    lw, win_d[kc * 128:(kc + 1) * 128, d2 * 128:(d2 + 1) * 128])
                nc.tensor.matmul(ps, lw, xnb[kc], start=(kc == 0), stop=(kc == KC - 1))
            if d2 < G:
                xi = gp.tile([128, T + 4], F32, tag=f"xi{d2}")
                nc.vector.tensor_copy(xi[:, 0:3], halo[d2][:, 0:3])
                nc.scalar.copy(xi[:, 3:3 + T], ps)
                nc.vector.tensor_copy(halo[d2][:, 0:3], xi[:, T:T + 3])
                xi_s.append(xi)
            else:
                g = d2 - G
                sg = convp.tile([128, T], BF16, tag="sg")
                nc.scalar.activation(sg, ps, AF.Sigmoid)
                sr = gp.tile([128, T], BF16, tag=f"sr{g}")
                nc.vector.tensor_mul(sr, ps, sg)
                sres.append(sr)
        # ---- conv + silu ----
        ci_b = []
        ps_xd = psB.tile([rpn, T], F32, tag="xd")
        for g in range(G):
            acc = convp.tile([128, T], F32, tag="acc")
            nc.vector.tensor_scalar_mul(acc, xi_s[g][:, 0:T], convw[:, 4 * g:4 * g + 1])
            for j in range(1, 4):
                acc2 = convp.tile([128, T], F32, tag="acc")
                nc.vector.scalar_tensor_tensor(
                    acc2, xi_s[g][:, j:j + T], convw[:, 4 * g + j:4 * g + j + 1],
                    acc, OP.mult, OP.add)
                acc = acc2
            sg2 = convp.tile([128, T], BF16, tag="sg")
            nc.scalar.activation(sg2, acc, AF.Sigmoid, bias=convb[:, g:g + 1])
            # silu(x+b) = (x+b)*sigmoid(x+b); add bias via tensor_scalar then mul
            accb = convp.tile([128, T], F32, tag="acc")
            nc.vector.tensor_scalar_add(accb, acc, convb[:, g:g + 1])
            cib = gp.tile([128, T], BF16, tag=f"ci{g}")
            nc.vector.tensor_mul(cib, accb, sg2)
            ci_b.append(cib)
            # ---- x_proj accumulate ----
            nc.tensor.matmul(ps_xd, wx_sb[:, g, :], cib,
                             start=(g == 0), stop=(g == G - 1))
        dtr = rowp.tile([r, T], BF16, tag="dtr")
        nc.scalar.copy(dtr, ps_xd[0:r, :])
        b_sb = rowp.tile([NST, T], BF16, tag="bsb")
        nc.scalar.copy(b_sb, ps_xd[b_off:b_off + NST, :])
        c_sb = rowp.tile([NST, T], BF16, tag="csb")
        nc.scalar.copy(c_sb, ps_xd[c_off:c_off + NST, :])
        nc.sync.dma_start(bc_scr[c, 0:NST], b_sb)
        nc.sync.dma_start(bc_scr[c, NST:2 * NST], c_sb)
        brep = bigp.tile([128, NST, T], BF16, tag="brep")
        crep = bigp.tile([128, NST, T], BF16, tag="crep")
        for n in range(NST):
            nc.sync.dma_start(brep[:, n, :], _bcast_ap(bc_scr[c], n, T))
            nc.sync.dma_start(crep[:, n, :], _bcast_ap(bc_scr[c], NST + n, T))
        # ---- dt / softplus / scan / gate per group ----
        ps_ss2 = psS.tile([1, T], F32, tag="ss2")
        ob = []
        for g in range(G):
            ps_dt = psA.tile([128, T], F32, tag="mm")
            nc.tensor.matmul(ps_dt, wdt_sb[:, g * 128:(g + 1) * 128], dtr,
                             start=True, stop=True)
            zc = convp.tile([128, T], F32, tag="acc")
            nc.vector.tensor_scalar(zc, ps_dt, dtb[:, g:g + 1], 30.0, OP.add, OP.min)
            ex = convp.tile([128, T], F32, tag="acc")
            nc.scalar.activation(ex, zc, AF.Exp)
            p1 = convp.tile([128, T], F32, tag="acc")
            nc.vector.tensor_scalar_add(p1, ex, 1.0)
            delta = gp.tile([128, T], F32, tag="delta")
            nc.scalar.activation(delta, p1, AF.Ln)
            du = gp.tile([128, T], BF16, tag="du")
            nc.vector.tensor_mul(du, delta, ci_b[g])
            h_t = bigp.tile([128, NST, T], BF16, tag="h")
            pr_t = bigp.tile([128, NST, T], BF16, tag="pr")
            for n in range(NST):
                da = scanp.tile([128, T], BF16, tag="da")
                nc.scalar.activation(da, delta, AF.Exp, scale=a_t[:, n:n + 1])
                db = scanp.tile([128, T], BF16, tag="db")
                nc.vector.tensor_mul(db, du, brep[:, n, :])
                init = 0.0 if c == 0 else hstate[g][:, n:n + 1]
                nc.vector.tensor_tensor_scan(
                    h_t[:, n, :], da, db, init, OP.mult, OP.add)
                nc.vector.tensor_mul(pr_t[:, n, :], h_t[:, n, :], crep[:, n, :])
            nc.vector.tensor_copy(hstate[g], h_t[:, :, T - 1])
            t1 = treep.tile([128, 8, T], BF16, tag="t1")
            nc.vector.tensor_add(t1, pr_t[:, 0:8, :], pr_t[:, 8:16, :])
            t2 = treep.tile([128, 4, T], BF16, tag="t2")
            nc.vector.tensor_add(t2, t1[:, 0:4, :], t1[:, 4:8, :])
            t3 = treep.tile([128, 2, T], BF16, tag="t3")
            nc.vector.tensor_add(t3, t2[:, 0:2, :], t2[:, 2:4, :])
            yg = convp.tile([128, T], F32, tag="acc")
            nc.vector.tensor_add(yg, t3[:, 0, :], t3[:, 1, :])
            yd = convp.tile([128, T], F32, tag="acc")
            nc.vector.scalar_tensor_tensor(
                yd, ci_b[g], dd[:, g:g + 1], yg, OP.mult, OP.add)
            o = gp.tile([128, T], BF16, tag=f"ob{g}")
            nc.vector.tensor_mul(o, yd, sres[g])
            ob.append(o)
        # ---- out_proj + residual + final norm ----
        h_res = []
        for kc in range(KC):
            ps_o = psA.tile([128, T], F32, tag="mm")
            for g in range(G):
                lw = lhsp.tile([128, 128], BF16, tag="lw")
                nc.sync.dma_start(
                    lw, wo_d[g * 128:(g + 1) * 128, kc * 128:(kc + 1) * 128])
                nc.tensor.matmul(ps_o, lw, ob[g], start=(g == 0), stop=(g == G - 1))
            hr = xp.tile([128, T], F32, tag=f"hr{kc}")
            nc.vector.tensor_add(hr, ps_o, x_t[kc])
            h_res.append(hr)
            hsq = gp.tile([128, T], F32, tag="xsq")
            nc.scalar.square(hsq, hr)
            nc.tensor.matmul(ps_ss2, ones_f, hsq, start=(kc == 0), stop=(kc == KC - 1))
        rrow2 = rowp.tile([1, T], F32, tag="rrow")
        nc.scalar.activation(rrow2, ps_ss2, AF.Sqrt, bias=epsc[:, 0:1], scale=1.0 / d)
        nc.vector.reciprocal(rrow2, rrow2)
        nc.sync.dma_start(rs_scr[c, 1:2, :], rrow2)
        rrep2 = rowp.tile([128, T], F32, tag="rrep")
        nc.sync.dma_start(rrep2, _bcast_ap(rs_scr[c], 1, T))
        for kc in range(KC):
            ot = xp.tile([128, T], F32, tag=f"ot{kc}")
            nc.vector.scalar_tensor_tensor(
                ot, h_res[kc], nfw[:, kc:kc + 1], rrep2, OP.mult, OP.mult)
            nc.sync.dma_start(o_d[kc * 128:(kc + 1) * 128, cs], ot)


_NC_CACHE = {}


def build_nc():
    if "nc" in _NC_CACHE:
        return _NC_CACHE["nc"]
    nc = bacc.Bacc("TRN2", target_bir_lowering=False, debug=False)
    with tile.TileContext(nc) as tc:
        for i, (d, H, W, T) in zip((3, 4, 5), SCALES):
            with ExitStack() as ctx:
                _scale_program(nc, tc, ctx, i, d, H * W, T)
    nc.compile()
    _NC_CACHE["nc"] = nc
    return nc


def host_inputs(s3, s4, s5, params3, params4, params5):
    """Per-core input dicts (host-side prep, numpy only)."""
    bf = ml_dtypes.bfloat16
    shared = {}
    for i, p, s in ((3, params3, s3), (4, params4, s4), (5, params5, s5)):
        d = s.shape[1]
        di, G, KC = 2 * d, 2 * d // 128, d // 128
        pn = {k: np.asarray(v, np.float32) for k, v in p.items()}
        shared[f"win{i}"] = (pn["in_proj"] * pn["norm_w"][:, None]).astype(bf)
        rr = -(-d // 16)
        b_off = ((rr + 31) // 32) * 32
        wxp = np.zeros((di, 128), np.float32)
        wxp[:, 0:rr] = pn["x_proj"][:, 0:rr]
        wxp[:, b_off:b_off + 16] = pn["x_proj"][:, rr:rr + 16]
        wxp[:, b_off + 32:b_off + 48] = pn["x_proj"][:, rr + 16:rr + 32]
        shared[f"wx{i}"] = wxp.astype(bf)
        shared[f"wdt{i}"] = pn["dt_w"].astype(bf)
        shared[f"wo{i}"] = pn["out_proj"].astype(bf)
        shared[f"convw{i}"] = np.ascontiguousarray(
            pn["conv_w"].reshape(G, 128, 4).transpose(1, 0, 2).reshape(128, G * 4))
        shared[f"convb{i}"] = np.ascontiguousarray(pn["conv_b"].reshape(G, 128).T)
        shared[f"dtb{i}"] = np.ascontiguousarray(pn["dt_b"].reshape(G, 128).T)
        shared[f"dd{i}"] = np.ascontiguousarray(pn["D"].reshape(G, 128).T)
        shared[f"nfw{i}"] = np.ascontiguousarray(pn["norm_f_w"].reshape(KC, 128).T)
        shared[f"av{i}"] = (-np.exp(pn["A_log"][0:1, :])).astype(np.float32)
    in_maps = []
    for b in range(NCORES):
        m = dict(shared)
        for i, s in ((3, s3), (4, s4), (5, s5)):
            a = np.asarray(s, np.float32)
            B, C, H, W = a.shape
            m[f"x{i}"] = np.ascontiguousarray(a[b].reshape(C, H * W))
        in_maps.append(m)
    return in_maps


def kernel(s3, s4, s5, params3, params4, params5):
    in_maps = host_inputs(s3, s4, s5, params3, params4, params5)
    nc = build_nc()
    res = run_bass_kernel_spmd(nc, in_maps, core_ids=list(range(NCORES)))
    outs = []
    for i, s in ((3, s3), (4, s4), (5, s5)):
        B, C, H, W = np.asarray(s).shape
        o = np.stack([res.results[b][f"o{i}"] for b in range(NCORES)])
        outs.append(o.reshape(B, C, H, W).astype(np.float32))
    return tuple(outs)
